# revision 58
# baseline (speedup 1.0000x reference)
"""Trainium2 Bass kernel for nn_BertAttentionEx (BERT attention with
relative_key_query position embeddings + output dense + residual + LayerNorm).

Distribution: 8 cores = 4 batches x 2 sequence-halves (data parallel over
query rows; K/V computed for the full sequence on each core). No collectives.

Per core: fp8 QKV projections, relative-position terms via dense band
matmuls in fp8 against pre-scaled distance tables (even/odd heads of a pair
run concurrently in the two PE row-groups via tile_position), skewed strided DMA
round trips through DRAM for the shear, regular identity-rhs matmuls for the
q-side band transpose (out = lhsT.T @ I into fp32 PSUM), transposed-softmax
(scores kept as s^T), v augmented with a ones-column so softmax normalizers
fall out of the PV matmul, fp8 output dense, then residual + LayerNorm in
fp32 on each core's 512 rows.

Scale folding: tables x8, q/k x8 (=> scores x64, exp scale 1/512), weights
x16, v x16, ctx2 = 16*ctx via ln-bias, Wo product x256 undone at PSUM copy.
"""
import sys
import math
import numpy as np
import ml_dtypes
from contextlib import ExitStack

sys.path.insert(0, "/opt/trn_rl_repo")

import concourse.bass as bass
import concourse.bacc as bacc
import concourse.tile as tile
from concourse import mybir
from concourse.bass_utils import run_bass_kernel_spmd

B, S, HID = 4, 1024, 1024
NH, HD = 16, 64
MAX_POS = 1024
LN_EPS = 1e-12
NCORES = 8
SL = 512          # query rows per core
WQ = 1152         # q-band width per 128-row tile
WK = 640          # k-band width per 128-row tile
F32 = mybir.dt.float32
F32R = mybir.dt.float32r
BF16 = mybir.dt.bfloat16
F8 = mybir.dt.float8e4
AF = mybir.ActivationFunctionType
ALU = mybir.AluOpType
DR = mybir.MatmulPerfMode.DoubleRow

USE_DOUBLE_ROW = False   # fp8 DoubleRow for QKV/Wo projections
FUSED_SCORES = True      # assemble scores in one PSUM accumulation group
SPLIT_C = False          # contiguous K=64 + row-group concurrency

_COMPILED = None


def r32(ap):
    return ap.bitcast(F32R)


def build_program():
    nc = bacc.Bacc("TRN2", target_bir_lowering=False, debug=False,
                   num_devices=NCORES)

    # ---- per-core external I/O ----
    hsT8d = nc.declare_dram_parameter("hsT8", [128, 8, S], F8, isOutput=False)
    hsQ8d = nc.declare_dram_parameter("hsQ8", [128, 8, SL], F8, isOutput=False)
    resd = nc.declare_dram_parameter("res", [SL, HID], F32, isOutput=False)
    wq8d = nc.declare_dram_parameter("wq8", [128, 8, 1024], F8, isOutput=False)
    wk8d = nc.declare_dram_parameter("wk8", [128, 8, 1024], F8, isOutput=False)
    wv8d = nc.declare_dram_parameter("wv8", [128, 8, 1040], F8, isOutput=False)
    wo8d = nc.declare_dram_parameter("wo8", [128, 8, 1024], F8, isOutput=False)
    bq8d = nc.declare_dram_parameter("bq8", [128, 8], F32, isOutput=False)
    bk8d = nc.declare_dram_parameter("bk8", [128, 8], F32, isOutput=False)
    bv16d = nc.declare_dram_parameter("bv16", [1040], F32, isOutput=False)
    drT8d = nc.declare_dram_parameter("drT8", [128, 2048], F8, isOutput=False)
    dT8d = nc.declare_dram_parameter("dT8", [128, 2048], F8, isOutput=False)
    id8d = nc.declare_dram_parameter("id8", [128, 128], F8, isOutput=False)
    maskd = nc.declare_dram_parameter("maskc", [128, 8], F32, isOutput=False)
    onesd = nc.declare_dram_parameter("ones64", [128, 64], F32R, isOutput=False)
    lngd = nc.declare_dram_parameter("lng", [HID], F32, isOutput=False)
    lnbd = nc.declare_dram_parameter("lnb", [HID], F32, isOutput=False)
    out = nc.declare_dram_parameter("out", [SL, HID], F32, isOutput=True)

    # SPMD = one program for all cores, so the band-table j0 formulas must be
    # core-independent: the distance tables are passed PRE-SHIFTED per core
    # (by that core's l0) so the kernel can use j0q = 896-128t, j0k = 896-128u.

    # internal DRAM: band buffers
    bq_dram = nc.dram_tensor("bq_dram", [NH, 4, 128, WQ], F8)
    bk_dram = nc.dram_tensor("bk_dram", [NH, 8, 128, WK], F8)

    with ExitStack() as ctx:
        tc = ctx.enter_context(tile.TileContext(nc))
        consts = ctx.enter_context(tc.tile_pool(name="consts", bufs=1))
        persist = ctx.enter_context(tc.tile_pool(name="persist", bufs=1))
        wpool = ctx.enter_context(tc.tile_pool(name="wpool", bufs=2))
        bigp = ctx.enter_context(tc.tile_pool(name="bigp", bufs=12))
        bandsb = ctx.enter_context(tc.tile_pool(name="bandsb", bufs=6))
        gqpool = ctx.enter_context(tc.tile_pool(name="gqpool", bufs=4))
        gkpool = ctx.enter_context(tc.tile_pool(name="gkpool", bufs=6))
        ppool = ctx.enter_context(tc.tile_pool(name="ppool", bufs=6))
        misc = ctx.enter_context(tc.tile_pool(name="misc", bufs=4))
        lns = ctx.enter_context(tc.tile_pool(name="lns", bufs=3))
        psP = ctx.enter_context(tc.tile_pool(name="psP", bufs=3, space="PSUM"))
        psQ = ctx.enter_context(tc.tile_pool(name="psQ", bufs=2, space="PSUM"))
        psCtx = ctx.enter_context(tc.tile_pool(name="psCtx", bufs=2, space="PSUM"))

        # ---- constants (small ones only; tables loaded after weights) ----
        id_sb = consts.tile([128, 128], F8)
        nc.sync.dma_start(out=id_sb, in_=id8d[:, :])
        bq_sb = consts.tile([128, 8], F32)
        nc.sync.dma_start(out=bq_sb, in_=bq8d[:, :])
        bk_sb = consts.tile([128, 8], F32)
        nc.sync.dma_start(out=bk_sb, in_=bk8d[:, :])
        mask_sb = consts.tile([128, 8], F32)
        nc.sync.dma_start(out=mask_sb, in_=maskd[:, :])
        ones_row = consts.tile([128, 64], F32R)
        nc.sync.dma_start(out=ones_row, in_=onesd[:, :])
        eps_sb = consts.tile([128, 1], F32)
        nc.vector.memset(eps_sb, LN_EPS)
        ln16_sb = consts.tile([128, 1], F32)
        nc.vector.memset(ln16_sb, math.log(16.0))

        # ---- persistent activations (per-chunk, Q-path inputs first so the
        # first projection matmuls start as early as possible) ----
        hsQ8 = persist.tile([128, 8, SL], F8, tag="hsQ8")
        wq_sb = wpool.tile([128, 8, 1024], F8, tag="w", name="wq_sb")
        for kc in range(8):
            nc.sync.dma_start(out=hsQ8[:, kc, :], in_=hsQ8d[:, kc, :])
            nc.sync.dma_start(out=wq_sb[:, kc, :], in_=wq8d[:, kc, :])
        hsT8 = persist.tile([128, 8, S], F8, tag="hsT8")
        for kc in range(8):
            nc.sync.dma_start(out=hsT8[:, kc, :], in_=hsT8d[:, kc, :])
        # q-dim (h,c) at partition 64*(h%2)+c, free slot h//2
        qT8 = persist.tile([128, 8, SL], F8, tag="qT8")
        kT8 = persist.tile([128, 8, S], F8, tag="kT8")
        vv8 = persist.tile([128, 8, 1040], F8, tag="vv8")  # [r', rtile, 65h+c]
        ctx2 = persist.tile([128, 8, SL], F8, tag="ctx2")  # 16*ctx/Z

        # ---- phase A: projections ----
        wk_sb = wpool.tile([128, 8, 1024], F8, tag="w", name="wk_sb")
        for kc in range(8):
            nc.sync.dma_start(out=wk_sb[:, kc, :], in_=wk8d[:, kc, :])
        # distance tables: needed only from phase B on
        drT_sb = consts.tile([128, 2048], F8)
        nc.sync.dma_start(out=drT_sb, in_=drT8d[:, :])
        dT_sb = consts.tile([128, 2048], F8)
        nc.sync.dma_start(out=dT_sb, in_=dT8d[:, :])

        def kchunks():
            # (slice-fn(tensor, colslice), start, stop, perf_mode) over K dim
            if USE_DOUBLE_ROW:
                return [(lambda w, cs, kp=kp: w[:, 2 * kp:2 * kp + 2, cs],
                         kp == 0, kp == 3, DR) for kp in range(4)]
            return [(lambda w, cs, kc=kc: w[:, kc, cs],
                     kc == 0, kc == 7, None) for kc in range(8)]

        for i in range(8):  # m-tile = heads (2i, 2i+1)
            ps = psP.tile([128, 512], F32, tag="ps", name=f"ps_q_{i}")
            for (sl, st, sp_, pm) in kchunks():
                nc.tensor.matmul(
                    ps,
                    lhsT=sl(wq_sb, slice(128 * i, 128 * i + 128)),
                    rhs=sl(hsQ8, slice(0, SL)),
                    start=st, stop=sp_, perf_mode=pm,
                )
            nc.scalar.activation(
                out=qT8[:, i, :], in_=ps, func=AF.Identity,
                bias=bq_sb[:, i:i + 1], scale=0.5,
            )
        for i in range(8):
            for ch in range(2):
                ps = psP.tile([128, 512], F32, tag="ps", name=f"ps_k_{i}_{ch}")
                for (sl, st, sp_, pm) in kchunks():
                    nc.tensor.matmul(
                        ps,
                        lhsT=sl(wk_sb, slice(128 * i, 128 * i + 128)),
                        rhs=sl(hsT8, slice(512 * ch, 512 * ch + 512)),
                        start=st, stop=sp_, perf_mode=pm,
                    )
                nc.scalar.activation(
                    out=kT8[:, i, 512 * ch:512 * ch + 512], in_=ps,
                    func=AF.Identity, bias=bk_sb[:, i:i + 1], scale=0.5,
                )
        wv_sb = wpool.tile([128, 8, 1040], F8, tag="w", name="wv_sb")
        nc.sync.dma_start(out=wv_sb, in_=wv8d[:, :, :])
        bv_bc = consts.tile([128, 1040], F32)
        nc.sync.dma_start(
            out=bv_bc,
            in_=bass.AP(tensor=bv16d, offset=0, ap=[[0, 128], [1, 1040]]),
        )
        for u in range(8):
            for (c0, cn) in ((0, 512), (512, 512), (1024, 16)):
                ps = psP.tile([128, 512], F32, tag="ps", name=f"ps_v_{u}_{c0}")
                for (sl, st, sp_, pm) in kchunks():
                    nc.tensor.matmul(
                        ps[:, 0:cn],
                        lhsT=sl(hsT8, slice(128 * u, 128 * u + 128)),
                        rhs=sl(wv_sb, slice(c0, c0 + cn)),
                        start=st, stop=sp_, perf_mode=pm,
                    )
                nc.vector.tensor_tensor(
                    out=vv8[:, u, c0:c0 + cn],
                    in0=ps[:, 0:cn], in1=bv_bc[:, c0:c0 + cn], op=ALU.add,
                )

        # ---- phase B: band matmuls -> DRAM (fp8) ----
        # q-band: bandq[h][t][l', j] = 8q[l'] . 8Drev2[j0q + j], j0q = 896-128t (host-shifted)
        # k-band: bandk[h][u][r', j] = 8k[r'] . 8D2[j0k + j],  j0k = 896-128u (host-shifted)
        # adjacent heads sit in different PE row-groups (a = h%4), so their
        # K=32x2 band matmuls run CONCURRENTLY when adjacent in the queue
        # with distinct tile_position row-groups and separate PSUM banks.
        for h0 in range(0, NH, 2):
            hpair = (h0, h0 + 1)
            for t in range(4):
                j0q = 896 - 128 * t
                bsbs = [bandsb.tile([128, WQ], F8, tag=f"bandq{i}",
                                    name=f"bq{h}_{t}")
                        for i, h in enumerate(hpair)]
                for (c0, cn) in ((0, 512), (512, 512), (1024, 128)):
                    pss = []
                    for i, h in enumerate(hpair):
                        a4, hq4 = 32 * (h % 4), h // 4
                        ps = psP.tile([128, 512], F32, tag=f"ps{i}",
                                      name=f"psbq{h}_{t}_{c0}")
                        nc.tensor.matmul(
                            ps[:, 0:cn],
                            lhsT=qT8[a4:a4 + 32, hq4, :, 128 * t:128 * t + 128],
                            rhs=drT_sb[a4:a4 + 32, :, j0q + c0:j0q + c0 + cn],
                            start=True, stop=True, perf_mode=DR,
                            tile_position=(a4, 0),
                        )
                        pss.append(ps)
                    for i in range(2):
                        nc.scalar.copy(out=bsbs[i][:, c0:c0 + cn],
                                       in_=pss[i][:, 0:cn])
                for i, h in enumerate(hpair):
                    nc.sync.dma_start(out=bq_dram[h, t, :, :], in_=bsbs[i])
            for u in range(8):
                j0k = 896 - 128 * u
                bsbs = [bandsb.tile([128, WK], F8, tag=f"bandk{i}",
                                    name=f"bk{h}_{u}")
                        for i, h in enumerate(hpair)]
                for (c0, cn) in ((0, 512), (512, 128)):
                    pss = []
                    for i, h in enumerate(hpair):
                        a4, hq4 = 32 * (h % 4), h // 4
                        ps = psP.tile([128, 512], F32, tag=f"ps{i}",
                                      name=f"psbk{h}_{u}_{c0}")
                        nc.tensor.matmul(
                            ps[:, 0:cn],
                            lhsT=kT8[a4:a4 + 32, hq4, :, 128 * u:128 * u + 128],
                            rhs=dT_sb[a4:a4 + 32, :, j0k + c0:j0k + c0 + cn],
                            start=True, stop=True, perf_mode=DR,
                            tile_position=(a4, 0),
                        )
                        pss.append(ps)
                    for i in range(2):
                        # balance PSUM->SBUF copy load across Vector/Scalar
                        if (u + i) % 2 == 0:
                            nc.vector.tensor_copy(out=bsbs[i][:, c0:c0 + cn],
                                                  in_=pss[i][:, 0:cn])
                        else:
                            nc.scalar.copy(out=bsbs[i][:, c0:c0 + cn],
                                           in_=pss[i][:, 0:cn])
                for i, h in enumerate(hpair):
                    nc.sync.dma_start(out=bk_dram[h, u, :, :], in_=bsbs[i])

        # phase D weights/consts issued here: transfers overlap phase C
        wo_sb = wpool.tile([128, 8, 1024], F8, tag="w", name="wo_sb")
        nc.sync.dma_start(out=wo_sb, in_=wo8d[:, :, :])
        lng_bc = consts.tile([128, HID], F32)
        nc.sync.dma_start(
            out=lng_bc,
            in_=bass.AP(tensor=lngd, offset=0, ap=[[0, 128], [1, HID]]),
        )
        lnb_bc = consts.tile([128, HID], F32)
        nc.sync.dma_start(
            out=lnb_bc,
            in_=bass.AP(tensor=lnbd, offset=0, ap=[[0, 128], [1, HID]]),
        )
        rsbs = []
        for lt in range(4):
            rsb = bigp.tile([128, HID], F32, tag="big", name=f"rsb{lt}")
            nc.sync.dma_start(out=rsb, in_=resd[128 * lt:128 * lt + 128, :])
            rsbs.append(rsb)

        # ---- phase C: attention per head ----
        # skewed row-gathers of the q-band: gqpre[t][l', r] (fp8);
        # issued one head ahead so the PE never waits at head boundaries
        def issue_gq(h):
            tiles = []
            for t in range(4):
                gq = gqpool.tile([128, S], F8, tag=f"gq{t}", name=f"gq{h}_{t}")
                nc.sync.dma_start(
                    out=gq,
                    in_=bass.AP(tensor=bq_dram,
                                offset=(h * 4 + t) * 128 * WQ + 127,
                                ap=[[WQ - 1, 128], [1, S]]),
                )
                tiles.append(gq)
            return tiles

        gq_next = issue_gq(0)
        for h in range(NH):
            hb, hp = 64 * (h % 2), h // 2
            a4, hq4 = 32 * (h % 4), h // 4
            gqpre = gq_next
            if h + 1 < NH:
                gq_next = issue_gq(h + 1)

            cps = psCtx.tile([65, 512], F32, tag="ctx", name=f"cps{h}")
            for u in range(8):
                # k-band skewed row-gather: gk[r', l'] (fp8)
                gk = gkpool.tile([128, SL], F8, tag="gk", name=f"gk{h}_{u}")
                nc.sync.dma_start(
                    out=gk,
                    in_=bass.AP(tensor=bk_dram,
                                offset=(h * 8 + u) * 128 * WK + 127,
                                ap=[[WK - 1, 128], [1, SL]]),
                )
                if FUSED_SCORES:
                    # scores assembled in one PSUM accumulation group:
                    # q-band blocks transposed-and-added via identity
                    # matmuls, k-band added via identity lhsT, QK^T on top.
                    sp = psP.tile([128, 512], F32, tag="ps", name=f"sp{h}_{u}")
                    for t in range(4):
                        nc.tensor.matmul(
                            sp[:, 128 * t:128 * t + 128],
                            lhsT=gqpre[t][:, 128 * u:128 * u + 128],
                            rhs=id_sb,
                            start=True, stop=False, skip_group_check=True,
                        )
                    nc.tensor.matmul(
                        sp, lhsT=id_sb, rhs=gk,
                        start=False, stop=False, skip_group_check=True,
                    )
                    nc.tensor.matmul(
                        sp,
                        lhsT=kT8[a4:a4 + 32, hq4, :, 128 * u:128 * u + 128],
                        rhs=qT8[a4:a4 + 32, hq4, :, :],
                        start=False, stop=True, skip_group_check=True,
                        perf_mode=DR, tile_position=(a4, 0),
                    )
                    sexp_in = sp
                else:
                    # conservative path: transposes to their own PSUM tile,
                    # sums on DVE
                    gqps = psQ.tile([128, 512], F32, tag="gqps", name=f"gqps{h}_{u}")
                    for t in range(4):
                        nc.tensor.matmul(
                            gqps[:, 128 * t:128 * t + 128],
                            lhsT=gqpre[t][:, 128 * u:128 * u + 128],
                            rhs=id_sb,
                            start=True, stop=True,
                        )
                    sp = psP.tile([128, 512], F32, tag="ps", name=f"sp{h}_{u}")
                    nc.tensor.matmul(
                        sp,
                        lhsT=kT8[a4:a4 + 32, hq4, :, 128 * u:128 * u + 128],
                        rhs=qT8[a4:a4 + 32, hq4, :, :],
                        start=True, stop=True,
                        perf_mode=DR, tile_position=(a4, 0),
                    )
                    gs = gkpool.tile([128, SL], BF16, tag="gs", name=f"gs{h}_{u}")
                    nc.vector.tensor_tensor(out=gs, in0=gqps, in1=gk, op=ALU.add)
                    ssb = gkpool.tile([128, SL], BF16, tag="ssb", name=f"ssb{h}_{u}")
                    nc.vector.tensor_tensor(out=ssb, in0=sp, in1=gs, op=ALU.add)
                    sexp_in = ssb
                pt = ppool.tile([128, SL], F8, tag="pt", name=f"pt{h}_{u}")
                nc.scalar.activation(
                    out=pt, in_=sexp_in, func=AF.Exp,
                    bias=mask_sb[:, u:u + 1], scale=1.0 / 512.0,
                )
                nc.tensor.matmul(
                    cps,
                    lhsT=vv8[:, u, 65 * h:65 * h + 65],
                    rhs=pt,
                    start=(u == 0), stop=(u == 7),
                )

            # softmax normalizer: Z on row 64; copy (x 1/16), broadcast via
            # ones-matmul, reciprocal on DVE (keeps Scalar's act table on Exp)
            zsb = misc.tile([128, 512], F32R, tag="zsb", name=f"zsb{h}")
            nc.vector.tensor_scalar(out=zsb[64:65, :], in0=cps[64:65, :],
                                    scalar1=1.0 / 16.0, scalar2=None,
                                    op0=ALU.mult)
            zps = psP.tile([64, 512], F32, tag="ps", name=f"zps{h}")
            nc.tensor.matmul(
                zps,
                lhsT=ones_row[64:65, :],
                rhs=zsb[64:65, :],
                start=True, stop=True,
            )
            zrec = misc.tile([64, 512], F32, tag="zrec", name=f"zrec{h}")
            nc.vector.reciprocal(out=zrec, in_=zps)
            nc.vector.tensor_tensor(
                out=ctx2[hb:hb + 64, hp, :],
                in0=cps[0:64, :], in1=zrec, op=ALU.mult,
            )

        # ---- phase D: output dense, residual, LayerNorm ----

        for lt in range(4):
            osb = bigp.tile([128, HID], F32, tag="big", name=f"osb{lt}")
            for mh in range(2):
                ps = psP.tile([128, 512], F32, tag="ps", name=f"ps_o_{lt}_{mh}")
                for (sl, st, sp_, pm) in kchunks():
                    nc.tensor.matmul(
                        ps,
                        lhsT=sl(ctx2, slice(128 * lt, 128 * lt + 128)),
                        rhs=sl(wo_sb, slice(512 * mh, 512 * mh + 512)),
                        start=st, stop=sp_, perf_mode=pm,
                    )
                nc.scalar.activation(
                    out=osb[:, 512 * mh:512 * mh + 512], in_=ps,
                    func=AF.Identity, scale=1.0 / 256.0,
                )
            h2 = bigp.tile([128, HID], F32, tag="big", name=f"h2_{lt}")
            nc.vector.tensor_tensor(out=h2, in0=osb, in1=rsbs[lt], op=ALU.add)

            stat = lns.tile([128, 16], F32, tag="stat", name=f"stat{lt}")
            for c in range(2):
                nc.vector.bn_stats(out=stat[:, 6 * c:6 * c + 6],
                                   in_=h2[:, 512 * c:512 * c + 512])
            mv = lns.tile([128, 4], F32, tag="mv", name=f"mv{lt}")
            nc.vector.bn_aggr(out=mv[:, 0:2],
                              in_=stat[:, 0:12].rearrange("p (n s) -> p n s", n=2))
            nc.scalar.activation(out=mv[:, 2:3], in_=mv[:, 1:2],
                                 func=AF.Sqrt, bias=eps_sb, scale=1.0)
            nc.vector.reciprocal(out=mv[:, 3:4], in_=mv[:, 2:3])

            xn = bigp.tile([128, HID], F32, tag="big", name=f"xn{lt}")
            nc.vector.tensor_scalar(
                out=xn, in0=h2,
                scalar1=mv[:, 0:1], scalar2=mv[:, 3:4],
                op0=ALU.subtract, op1=ALU.mult,
            )
            xg = bigp.tile([128, HID], F32, tag="big", name=f"xg{lt}")
            nc.vector.tensor_tensor(out=xg, in0=xn, in1=lng_bc, op=ALU.mult)
            ob = bigp.tile([128, HID], F32, tag="big", name=f"ob{lt}")
            nc.vector.tensor_tensor(out=ob, in0=xg, in1=lnb_bc, op=ALU.add)
            nc.sync.dma_start(out=out[128 * lt:128 * lt + 128, :], in_=ob)

    nc.compile()
    return nc


def make_in_maps(hidden_states, attention_mask, Wq, bq, Wk, bk, Wv, bv,
                 dist_emb, Wo, bo, ln_g, ln_b):
    E4 = ml_dtypes.float8_e4m3
    hs = np.asarray(hidden_states, np.float32)
    mask = np.asarray(attention_mask, np.float32)
    Wq = np.asarray(Wq, np.float32); Wk = np.asarray(Wk, np.float32)
    Wv = np.asarray(Wv, np.float32); Wo = np.asarray(Wo, np.float32)
    bq = np.asarray(bq, np.float32); bk = np.asarray(bk, np.float32)
    bv = np.asarray(bv, np.float32); bo = np.asarray(bo, np.float32)
    D = np.asarray(dist_emb, np.float32)
    ln_g = np.asarray(ln_g, np.float32); ln_b = np.asarray(ln_b, np.float32)

    # padded tables [2048, 64]
    z1 = np.zeros((1, HD), np.float32)
    D2 = np.concatenate([D, z1], 0)          # D2[x] = D[x], x<=2046
    Dr2 = np.concatenate([D[::-1], z1], 0)   # Dr2[i] = D[2046-i], i<=2046

    # weights in PE layout [128, 8, M]: w[p, kc, m] = 16*W[m, 128*kc + p]
    def wlay(W):  # W: [M, 1024]
        return np.ascontiguousarray(
            (16.0 * W.T).reshape(8, 128, W.shape[0]).transpose(1, 0, 2)
        ).astype(E4)

    wq8 = wlay(Wq)
    wk8 = wlay(Wk)
    wo8 = wlay(Wo)

    # augmented V weights: cols 65h+c = 16*Wv[64h+c, :], col 65h+64 = 0
    WvA = np.zeros((1040, HID), np.float32)
    bvA = np.zeros(1040, np.float32)
    for h in range(NH):
        WvA[65 * h:65 * h + 64] = 16.0 * Wv[64 * h:64 * h + 64]
        bvA[65 * h:65 * h + 64] = 16.0 * bv[64 * h:64 * h + 64]
        bvA[65 * h + 64] = 16.0
    wv8 = np.ascontiguousarray(
        WvA.T.reshape(8, 128, 1040).transpose(1, 0, 2)).astype(E4)

    id8 = np.eye(128, dtype=np.float32).astype(E4)
    ones64 = np.ones((128, 64), np.float32)

    in_maps = []
    for core in range(NCORES):
        b, g = core // 2, core % 2
        l0 = SL * g
        # tables host-shifted by l0 so the kernel's j0 formulas are
        # core-independent: kernel reads drT8[:, (896-128t)+j]; the true
        # offset is 896-l0-128t => shift the reversed table left by l0.
        # dT8: kernel reads dT8[:, (896-128u)+j]; true j0k = l0+896-128u
        # => shift D2 right by -l0 i.e. index + l0.
        # want drT8[jk] = Dr2[jk - l0]  (since true j0q = 896-l0-128t)
        drT = np.zeros((2048, HD), np.float32)
        if l0 == 0:
            drT[:] = Dr2
        else:
            drT[l0:] = Dr2[:2048 - l0]
        dT = np.zeros((2048, HD), np.float32)
        # want dT8[jk] = D2[jk + l0]  (true j0k = l0+896-128u)
        if l0 == 0:
            dT[:] = D2
        else:
            dT[:2048 - l0] = D2[l0:]
        drT8 = np.ascontiguousarray(
            np.tile((8.0 * drT).T, (2, 1))).astype(E4)   # [128, 2048]
        dT8 = np.ascontiguousarray(
            np.tile((8.0 * dT).T, (2, 1))).astype(E4)

        hsT8 = np.ascontiguousarray(
            hs[b].T.reshape(8, 128, S).transpose(1, 0, 2)).astype(E4)
        hsQ8 = np.ascontiguousarray(hsT8[:, :, l0:l0 + SL])
        res = np.ascontiguousarray(hs[b, l0:l0 + SL] + bo[None, :])
        in_maps.append({
            "hsT8": hsT8,
            "hsQ8": hsQ8,
            "res": res,
            "wq8": wq8, "wk8": wk8, "wv8": wv8, "wo8": wo8,
            "bq8": np.ascontiguousarray((8.0 * bq).reshape(8, 128).T),
            "bk8": np.ascontiguousarray((8.0 * bk).reshape(8, 128).T),
            "bv16": bvA,
            "drT8": drT8, "dT8": dT8,
            "id8": id8,
            "maskc": np.ascontiguousarray(mask[b, 0, 0].reshape(8, 128).T),
            "ones64": ones64,
            "lng": ln_g, "lnb": ln_b,
        })
    return in_maps


def kernel(**inputs):
    global _COMPILED
    if _COMPILED is None:
        _COMPILED = build_program()
    nc = _COMPILED
    in_maps = make_in_maps(**inputs)
    result = run_bass_kernel_spmd(nc, in_maps, core_ids=list(range(NCORES)))
    out = np.zeros((B, S, HID), np.float32)
    for core in range(NCORES):
        b, g = core // 2, core % 2
        out[b, SL * g:SL * g + SL] = result.results[core]["out"]
    return out


# revision 59
# speedup vs baseline: 1.2430x; 1.2430x over previous
"""Trainium2 Bass kernel for nn_BertAttentionEx (BERT attention with
relative_key_query position embeddings + output dense + residual + LayerNorm).

Distribution: 8 cores = 4 batches x 2 sequence-halves (data parallel over
query rows; K/V computed for the full sequence on each core). No collectives.

Per core: fp8 QKV projections, relative-position terms via dense band
matmuls in fp8 against pre-scaled distance tables (even/odd heads of a pair
run concurrently in the two PE row-groups via tile_position), skewed strided DMA
round trips through DRAM for the shear, regular identity-rhs matmuls for the
q-side band transpose (out = lhsT.T @ I into fp32 PSUM), transposed-softmax
(scores kept as s^T), v augmented with a ones-column so softmax normalizers
fall out of the PV matmul, fp8 output dense, then residual + LayerNorm in
fp32 on each core's 512 rows.

Scale folding: tables x8, q/k x8 (=> scores x64, exp scale 1/512), weights
x16, v x16, ctx2 = 16*ctx via ln-bias, Wo product x256 undone at PSUM copy.
"""
import sys
import math
import numpy as np
import ml_dtypes
from contextlib import ExitStack

sys.path.insert(0, "/opt/trn_rl_repo")

import concourse.bass as bass
import concourse.bacc as bacc
import concourse.tile as tile
from concourse import mybir
from concourse.bass_utils import run_bass_kernel_spmd

B, S, HID = 4, 1024, 1024
NH, HD = 16, 64
MAX_POS = 1024
LN_EPS = 1e-12
NCORES = 8
SL = 512          # query rows per core
WQ = 1152         # q-band width per 128-row tile
WK = 640          # k-band width per 128-row tile
F32 = mybir.dt.float32
F32R = mybir.dt.float32r
BF16 = mybir.dt.bfloat16
F8 = mybir.dt.float8e4
AF = mybir.ActivationFunctionType
ALU = mybir.AluOpType
DR = mybir.MatmulPerfMode.DoubleRow

USE_DOUBLE_ROW = False   # fp8 DoubleRow for QKV/Wo projections
FUSED_SCORES = True      # assemble scores in one PSUM accumulation group
SPLIT_C = False          # contiguous K=64 + row-group concurrency

_COMPILED = None


def r32(ap):
    return ap.bitcast(F32R)


def build_program():
    nc = bacc.Bacc("TRN2", target_bir_lowering=False, debug=False,
                   num_devices=NCORES)

    # ---- per-core external I/O ----
    hsT8d = nc.declare_dram_parameter("hsT8", [128, 8, S], F8, isOutput=False)
    hsQ8d = nc.declare_dram_parameter("hsQ8", [128, 8, SL], F8, isOutput=False)
    resd = nc.declare_dram_parameter("res", [SL, HID], F32, isOutput=False)
    wq8d = nc.declare_dram_parameter("wq8", [128, 8, 1024], F8, isOutput=False)
    wk8d = nc.declare_dram_parameter("wk8", [128, 8, 1024], F8, isOutput=False)
    wv8d = nc.declare_dram_parameter("wv8", [128, 8, 1040], F8, isOutput=False)
    wo8d = nc.declare_dram_parameter("wo8", [128, 8, 1024], F8, isOutput=False)
    bq8d = nc.declare_dram_parameter("bq8", [128, 8], F32, isOutput=False)
    bk8d = nc.declare_dram_parameter("bk8", [128, 8], F32, isOutput=False)
    bv16d = nc.declare_dram_parameter("bv16", [1040], F32, isOutput=False)
    drT8d = nc.declare_dram_parameter("drT8", [128, 2048], F8, isOutput=False)
    dT8d = nc.declare_dram_parameter("dT8", [128, 2048], F8, isOutput=False)
    id8d = nc.declare_dram_parameter("id8", [128, 128], F8, isOutput=False)
    maskd = nc.declare_dram_parameter("maskc", [128, 8], F32, isOutput=False)
    onesd = nc.declare_dram_parameter("ones64", [128, 64], F32R, isOutput=False)
    lngd = nc.declare_dram_parameter("lng", [HID], F32, isOutput=False)
    lnbd = nc.declare_dram_parameter("lnb", [HID], F32, isOutput=False)
    out = nc.declare_dram_parameter("out", [SL, HID], F32, isOutput=True)

    # SPMD = one program for all cores, so the band-table j0 formulas must be
    # core-independent: the distance tables are passed PRE-SHIFTED per core
    # (by that core's l0) so the kernel can use j0q = 896-128t, j0k = 896-128u.

    # internal DRAM: band buffers
    bq_dram = nc.dram_tensor("bq_dram", [NH, 4, 128, WQ], F8)
    bk_dram = nc.dram_tensor("bk_dram", [NH, 8, 128, WK], F8)

    with ExitStack() as ctx:
        tc = ctx.enter_context(tile.TileContext(nc))
        consts = ctx.enter_context(tc.tile_pool(name="consts", bufs=1))
        persist = ctx.enter_context(tc.tile_pool(name="persist", bufs=1))
        wpool = ctx.enter_context(tc.tile_pool(name="wpool", bufs=2))
        bigp = ctx.enter_context(tc.tile_pool(name="bigp", bufs=12))
        bandsb = ctx.enter_context(tc.tile_pool(name="bandsb", bufs=4))
        gqpool = ctx.enter_context(tc.tile_pool(name="gqpool", bufs=3))
        gkpool = ctx.enter_context(tc.tile_pool(name="gkpool", bufs=4))
        ppool = ctx.enter_context(tc.tile_pool(name="ppool", bufs=4))
        misc = ctx.enter_context(tc.tile_pool(name="misc", bufs=3))
        lns = ctx.enter_context(tc.tile_pool(name="lns", bufs=2))
        psP = ctx.enter_context(tc.tile_pool(name="psP", bufs=3, space="PSUM"))
        psQ = ctx.enter_context(tc.tile_pool(name="psQ", bufs=2, space="PSUM"))
        psCtx = ctx.enter_context(tc.tile_pool(name="psCtx", bufs=2, space="PSUM"))

        # ---- constants (small ones only; tables loaded after weights) ----
        id_sb = consts.tile([128, 128], F8)
        nc.sync.dma_start(out=id_sb, in_=id8d[:, :])
        bq_sb = consts.tile([128, 8], F32)
        nc.sync.dma_start(out=bq_sb, in_=bq8d[:, :])
        bk_sb = consts.tile([128, 8], F32)
        nc.sync.dma_start(out=bk_sb, in_=bk8d[:, :])
        mask_sb = consts.tile([128, 8], F32)
        nc.sync.dma_start(out=mask_sb, in_=maskd[:, :])
        ones_row = consts.tile([128, 64], F32R)
        nc.sync.dma_start(out=ones_row, in_=onesd[:, :])
        eps_sb = consts.tile([128, 1], F32)
        nc.vector.memset(eps_sb, LN_EPS)
        ln16_sb = consts.tile([128, 1], F32)
        nc.vector.memset(ln16_sb, math.log(16.0))

        # ---- persistent activations (per-chunk, Q-path inputs first so the
        # first projection matmuls start as early as possible) ----
        hsQ8 = persist.tile([128, 8, SL], F8, tag="hsQ8")
        wq_sb = wpool.tile([128, 8, 1024], F8, tag="w", name="wq_sb")
        for kc in range(8):
            nc.sync.dma_start(out=hsQ8[:, kc, :], in_=hsQ8d[:, kc, :])
            nc.sync.dma_start(out=wq_sb[:, kc, :], in_=wq8d[:, kc, :])
        hsT8 = persist.tile([128, 8, S], F8, tag="hsT8")
        for kc in range(8):
            nc.sync.dma_start(out=hsT8[:, kc, :], in_=hsT8d[:, kc, :])
        # q-dim (h,c) at partition 64*(h%2)+c, free slot h//2
        qT8 = persist.tile([128, 8, SL], F8, tag="qT8")
        kT8 = persist.tile([128, 8, S], F8, tag="kT8")
        vv8 = persist.tile([128, 8, 1040], F8, tag="vv8")  # [r', rtile, 65h+c]
        ctx2 = persist.tile([128, 8, SL], F8, tag="ctx2")  # 16*ctx/Z

        # ---- phase A: projections ----
        wk_sb = wpool.tile([128, 8, 1024], F8, tag="w", name="wk_sb")
        for kc in range(8):
            nc.sync.dma_start(out=wk_sb[:, kc, :], in_=wk8d[:, kc, :])
        # distance tables: needed only from phase B on
        drT_sb = consts.tile([128, 2048], F8)
        nc.sync.dma_start(out=drT_sb, in_=drT8d[:, :])
        dT_sb = consts.tile([128, 2048], F8)
        nc.sync.dma_start(out=dT_sb, in_=dT8d[:, :])

        def kchunks():
            # (slice-fn(tensor, colslice), start, stop, perf_mode) over K dim
            if USE_DOUBLE_ROW:
                return [(lambda w, cs, kp=kp: w[:, 2 * kp:2 * kp + 2, cs],
                         kp == 0, kp == 3, DR) for kp in range(4)]
            return [(lambda w, cs, kc=kc: w[:, kc, cs],
                     kc == 0, kc == 7, None) for kc in range(8)]

        for i in range(8):  # m-tile = heads (2i, 2i+1)
            ps = psP.tile([128, 512], F32, tag="ps", name=f"ps_q_{i}")
            for (sl, st, sp_, pm) in kchunks():
                nc.tensor.matmul(
                    ps,
                    lhsT=sl(wq_sb, slice(128 * i, 128 * i + 128)),
                    rhs=sl(hsQ8, slice(0, SL)),
                    start=st, stop=sp_, perf_mode=pm,
                )
            nc.scalar.activation(
                out=qT8[:, i, :], in_=ps, func=AF.Identity,
                bias=bq_sb[:, i:i + 1], scale=0.5,
            )
        for i in range(8):
            for ch in range(2):
                ps = psP.tile([128, 512], F32, tag="ps", name=f"ps_k_{i}_{ch}")
                for (sl, st, sp_, pm) in kchunks():
                    nc.tensor.matmul(
                        ps,
                        lhsT=sl(wk_sb, slice(128 * i, 128 * i + 128)),
                        rhs=sl(hsT8, slice(512 * ch, 512 * ch + 512)),
                        start=st, stop=sp_, perf_mode=pm,
                    )
                nc.scalar.activation(
                    out=kT8[:, i, 512 * ch:512 * ch + 512], in_=ps,
                    func=AF.Identity, bias=bk_sb[:, i:i + 1], scale=0.5,
                )
        wv_sb = wpool.tile([128, 8, 1040], F8, tag="w", name="wv_sb")
        nc.sync.dma_start(out=wv_sb, in_=wv8d[:, :, :])
        bv_bc = consts.tile([128, 1040], F32)
        nc.sync.dma_start(
            out=bv_bc,
            in_=bass.AP(tensor=bv16d, offset=0, ap=[[0, 128], [1, 1040]]),
        )
        for u in range(8):
            for (c0, cn) in ((0, 512), (512, 512), (1024, 16)):
                ps = psP.tile([128, 512], F32, tag="ps", name=f"ps_v_{u}_{c0}")
                for (sl, st, sp_, pm) in kchunks():
                    nc.tensor.matmul(
                        ps[:, 0:cn],
                        lhsT=sl(hsT8, slice(128 * u, 128 * u + 128)),
                        rhs=sl(wv_sb, slice(c0, c0 + cn)),
                        start=st, stop=sp_, perf_mode=pm,
                    )
                nc.vector.tensor_tensor(
                    out=vv8[:, u, c0:c0 + cn],
                    in0=ps[:, 0:cn], in1=bv_bc[:, c0:c0 + cn], op=ALU.add,
                )

        # ---- phase B: band matmuls -> DRAM (fp8) ----
        # q-band: bandq[h][t][l', j] = 8q[l'] . 8Drev2[j0q + j], j0q = 896-128t (host-shifted)
        # k-band: bandk[h][u][r', j] = 8k[r'] . 8D2[j0k + j],  j0k = 896-128u (host-shifted)
        # adjacent heads sit in different PE row-groups (a = h%4), so their
        # K=32x2 band matmuls run CONCURRENTLY when adjacent in the queue
        # with distinct tile_position row-groups and separate PSUM banks.
        for h0 in range(0, NH, 2):
            hpair = (h0, h0 + 1)
            for t in range(4):
                j0q = 896 - 128 * t
                bsbs = [bandsb.tile([128, WQ], F8, tag=f"bandq{i}",
                                    name=f"bq{h}_{t}")
                        for i, h in enumerate(hpair)]
                for (c0, cn) in ((0, 512), (512, 512), (1024, 128)):
                    pss = []
                    for i, h in enumerate(hpair):
                        a4, hq4 = 32 * (h % 4), h // 4
                        ps = psP.tile([128, 512], F32, tag=f"ps{i}",
                                      name=f"psbq{h}_{t}_{c0}")
                        nc.tensor.matmul(
                            ps[:, 0:cn],
                            lhsT=qT8[a4:a4 + 32, hq4, :, 128 * t:128 * t + 128],
                            rhs=drT_sb[a4:a4 + 32, :, j0q + c0:j0q + c0 + cn],
                            start=True, stop=True, perf_mode=DR,
                            tile_position=(a4, 0),
                        )
                        pss.append(ps)
                    for i in range(2):
                        nc.scalar.copy(out=bsbs[i][:, c0:c0 + cn],
                                       in_=pss[i][:, 0:cn])
                for i, h in enumerate(hpair):
                    nc.sync.dma_start(out=bq_dram[h, t, :, :], in_=bsbs[i])
            for u in range(8):
                j0k = 896 - 128 * u
                bsbs = [bandsb.tile([128, WK], F8, tag=f"bandk{i}",
                                    name=f"bk{h}_{u}")
                        for i, h in enumerate(hpair)]
                for (c0, cn) in ((0, 512), (512, 128)):
                    pss = []
                    for i, h in enumerate(hpair):
                        a4, hq4 = 32 * (h % 4), h // 4
                        ps = psP.tile([128, 512], F32, tag=f"ps{i}",
                                      name=f"psbk{h}_{u}_{c0}")
                        nc.tensor.matmul(
                            ps[:, 0:cn],
                            lhsT=kT8[a4:a4 + 32, hq4, :, 128 * u:128 * u + 128],
                            rhs=dT_sb[a4:a4 + 32, :, j0k + c0:j0k + c0 + cn],
                            start=True, stop=True, perf_mode=DR,
                            tile_position=(a4, 0),
                        )
                        pss.append(ps)
                    for i in range(2):
                        # balance PSUM->SBUF copy load across Vector/Scalar
                        if (u + i) % 2 == 0:
                            nc.vector.tensor_copy(out=bsbs[i][:, c0:c0 + cn],
                                                  in_=pss[i][:, 0:cn])
                        else:
                            nc.scalar.copy(out=bsbs[i][:, c0:c0 + cn],
                                           in_=pss[i][:, 0:cn])
                for i, h in enumerate(hpair):
                    nc.sync.dma_start(out=bk_dram[h, u, :, :], in_=bsbs[i])

        # phase D weights/consts issued here: transfers overlap phase C
        wo_sb = wpool.tile([128, 8, 1024], F8, tag="w", name="wo_sb")
        nc.sync.dma_start(out=wo_sb, in_=wo8d[:, :, :])
        lng_bc = consts.tile([128, HID], F32)
        nc.sync.dma_start(
            out=lng_bc,
            in_=bass.AP(tensor=lngd, offset=0, ap=[[0, 128], [1, HID]]),
        )
        lnb_bc = consts.tile([128, HID], F32)
        nc.sync.dma_start(
            out=lnb_bc,
            in_=bass.AP(tensor=lnbd, offset=0, ap=[[0, 128], [1, HID]]),
        )
        rsbs = []
        for lt in range(4):
            rsb = bigp.tile([128, HID], F32, tag="big", name=f"rsb{lt}")
            nc.sync.dma_start(out=rsb, in_=resd[128 * lt:128 * lt + 128, :])
            rsbs.append(rsb)

        # ---- phase C: attention per head ----
        # skewed row-gathers of the q-band: gqpre[t][l', r] (fp8);
        # issued one head ahead so the PE never waits at head boundaries
        def issue_gq(h):
            tiles = []
            for t in range(4):
                gq = gqpool.tile([128, S], F8, tag=f"gq{t}", name=f"gq{h}_{t}")
                nc.sync.dma_start(
                    out=gq,
                    in_=bass.AP(tensor=bq_dram,
                                offset=(h * 4 + t) * 128 * WQ + 127,
                                ap=[[WQ - 1, 128], [1, S]]),
                )
                tiles.append(gq)
            return tiles

        gq_next = issue_gq(0)
        for h in range(NH):
            hb, hp = 64 * (h % 2), h // 2
            a4, hq4 = 32 * (h % 4), h // 4
            gqpre = gq_next
            if h + 1 < NH:
                gq_next = issue_gq(h + 1)

            cps = psCtx.tile([65, 512], F32, tag="ctx", name=f"cps{h}")
            for u in range(8):
                # k-band skewed row-gather: gk[r', l'] (fp8)
                gk = gkpool.tile([128, SL], F8, tag="gk", name=f"gk{h}_{u}")
                nc.sync.dma_start(
                    out=gk,
                    in_=bass.AP(tensor=bk_dram,
                                offset=(h * 8 + u) * 128 * WK + 127,
                                ap=[[WK - 1, 128], [1, SL]]),
                )
                if FUSED_SCORES:
                    # scores assembled in one PSUM accumulation group:
                    # q-band blocks transposed-and-added via identity
                    # matmuls, k-band added via identity lhsT, QK^T on top.
                    sp = psP.tile([128, 512], F32, tag="ps", name=f"sp{h}_{u}")
                    for t in range(4):
                        nc.tensor.matmul(
                            sp[:, 128 * t:128 * t + 128],
                            lhsT=gqpre[t][:, 128 * u:128 * u + 128],
                            rhs=id_sb,
                            start=True, stop=False, skip_group_check=True,
                        )
                    nc.tensor.matmul(
                        sp, lhsT=id_sb, rhs=gk,
                        start=False, stop=False, skip_group_check=True,
                    )
                    nc.tensor.matmul(
                        sp,
                        lhsT=kT8[a4:a4 + 32, hq4, :, 128 * u:128 * u + 128],
                        rhs=qT8[a4:a4 + 32, hq4, :, :],
                        start=False, stop=True, skip_group_check=True,
                        perf_mode=DR, tile_position=(a4, 0),
                    )
                    sexp_in = sp
                else:
                    # conservative path: transposes to their own PSUM tile,
                    # sums on DVE
                    gqps = psQ.tile([128, 512], F32, tag="gqps", name=f"gqps{h}_{u}")
                    for t in range(4):
                        nc.tensor.matmul(
                            gqps[:, 128 * t:128 * t + 128],
                            lhsT=gqpre[t][:, 128 * u:128 * u + 128],
                            rhs=id_sb,
                            start=True, stop=True,
                        )
                    sp = psP.tile([128, 512], F32, tag="ps", name=f"sp{h}_{u}")
                    nc.tensor.matmul(
                        sp,
                        lhsT=kT8[a4:a4 + 32, hq4, :, 128 * u:128 * u + 128],
                        rhs=qT8[a4:a4 + 32, hq4, :, :],
                        start=True, stop=True,
                        perf_mode=DR, tile_position=(a4, 0),
                    )
                    gs = gkpool.tile([128, SL], BF16, tag="gs", name=f"gs{h}_{u}")
                    nc.vector.tensor_tensor(out=gs, in0=gqps, in1=gk, op=ALU.add)
                    ssb = gkpool.tile([128, SL], BF16, tag="ssb", name=f"ssb{h}_{u}")
                    nc.vector.tensor_tensor(out=ssb, in0=sp, in1=gs, op=ALU.add)
                    sexp_in = ssb
                pt = ppool.tile([128, SL], F8, tag="pt", name=f"pt{h}_{u}")
                nc.scalar.activation(
                    out=pt, in_=sexp_in, func=AF.Exp,
                    bias=mask_sb[:, u:u + 1], scale=1.0 / 512.0,
                )
                nc.tensor.matmul(
                    cps,
                    lhsT=vv8[:, u, 65 * h:65 * h + 65],
                    rhs=pt,
                    start=(u == 0), stop=(u == 7),
                )

            # softmax normalizer: Z on row 64; copy (x 1/16), broadcast via
            # ones-matmul, reciprocal on DVE (keeps Scalar's act table on Exp)
            zsb = misc.tile([128, 512], F32R, tag="zsb", name=f"zsb{h}")
            nc.vector.tensor_scalar(out=zsb[64:65, :], in0=cps[64:65, :],
                                    scalar1=1.0 / 16.0, scalar2=None,
                                    op0=ALU.mult)
            zps = psP.tile([64, 512], F32, tag="ps", name=f"zps{h}")
            nc.tensor.matmul(
                zps,
                lhsT=ones_row[64:65, :],
                rhs=zsb[64:65, :],
                start=True, stop=True,
            )
            zrec = misc.tile([64, 512], F32, tag="zrec", name=f"zrec{h}")
            nc.vector.reciprocal(out=zrec, in_=zps)
            nc.vector.tensor_tensor(
                out=ctx2[hb:hb + 64, hp, :],
                in0=cps[0:64, :], in1=zrec, op=ALU.mult,
            )

        # ---- phase D: output dense, residual, LayerNorm ----

        for lt in range(4):
            osb = bigp.tile([128, HID], F32, tag="big", name=f"osb{lt}")
            for mh in range(2):
                ps = psP.tile([128, 512], F32, tag="ps", name=f"ps_o_{lt}_{mh}")
                for (sl, st, sp_, pm) in kchunks():
                    nc.tensor.matmul(
                        ps,
                        lhsT=sl(ctx2, slice(128 * lt, 128 * lt + 128)),
                        rhs=sl(wo_sb, slice(512 * mh, 512 * mh + 512)),
                        start=st, stop=sp_, perf_mode=pm,
                    )
                nc.scalar.activation(
                    out=osb[:, 512 * mh:512 * mh + 512], in_=ps,
                    func=AF.Identity, scale=1.0 / 256.0,
                )
            h2 = bigp.tile([128, HID], F32, tag="big", name=f"h2_{lt}")
            nc.vector.tensor_tensor(out=h2, in0=osb, in1=rsbs[lt], op=ALU.add)

            stat = lns.tile([128, 16], F32, tag="stat", name=f"stat{lt}")
            for c in range(2):
                nc.vector.bn_stats(out=stat[:, 6 * c:6 * c + 6],
                                   in_=h2[:, 512 * c:512 * c + 512])
            mv = lns.tile([128, 4], F32, tag="mv", name=f"mv{lt}")
            nc.vector.bn_aggr(out=mv[:, 0:2],
                              in_=stat[:, 0:12].rearrange("p (n s) -> p n s", n=2))
            nc.scalar.activation(out=mv[:, 2:3], in_=mv[:, 1:2],
                                 func=AF.Sqrt, bias=eps_sb, scale=1.0)
            nc.vector.reciprocal(out=mv[:, 3:4], in_=mv[:, 2:3])

            xn = bigp.tile([128, HID], F32, tag="big", name=f"xn{lt}")
            nc.vector.tensor_scalar(
                out=xn, in0=h2,
                scalar1=mv[:, 0:1], scalar2=mv[:, 3:4],
                op0=ALU.subtract, op1=ALU.mult,
            )
            xg = bigp.tile([128, HID], F32, tag="big", name=f"xg{lt}")
            nc.vector.tensor_tensor(out=xg, in0=xn, in1=lng_bc, op=ALU.mult)
            ob = bigp.tile([128, HID], F32, tag="big", name=f"ob{lt}")
            nc.vector.tensor_tensor(out=ob, in0=xg, in1=lnb_bc, op=ALU.add)
            nc.sync.dma_start(out=out[128 * lt:128 * lt + 128, :], in_=ob)

    nc.compile()
    return nc


def make_in_maps(hidden_states, attention_mask, Wq, bq, Wk, bk, Wv, bv,
                 dist_emb, Wo, bo, ln_g, ln_b):
    E4 = ml_dtypes.float8_e4m3
    hs = np.asarray(hidden_states, np.float32)
    mask = np.asarray(attention_mask, np.float32)
    Wq = np.asarray(Wq, np.float32); Wk = np.asarray(Wk, np.float32)
    Wv = np.asarray(Wv, np.float32); Wo = np.asarray(Wo, np.float32)
    bq = np.asarray(bq, np.float32); bk = np.asarray(bk, np.float32)
    bv = np.asarray(bv, np.float32); bo = np.asarray(bo, np.float32)
    D = np.asarray(dist_emb, np.float32)
    ln_g = np.asarray(ln_g, np.float32); ln_b = np.asarray(ln_b, np.float32)

    # padded tables [2048, 64]
    z1 = np.zeros((1, HD), np.float32)
    D2 = np.concatenate([D, z1], 0)          # D2[x] = D[x], x<=2046
    Dr2 = np.concatenate([D[::-1], z1], 0)   # Dr2[i] = D[2046-i], i<=2046

    # weights in PE layout [128, 8, M]: w[p, kc, m] = 16*W[m, 128*kc + p]
    def wlay(W):  # W: [M, 1024]
        return np.ascontiguousarray(
            (16.0 * W.T).reshape(8, 128, W.shape[0]).transpose(1, 0, 2)
        ).astype(E4)

    wq8 = wlay(Wq)
    wk8 = wlay(Wk)
    wo8 = wlay(Wo)

    # augmented V weights: cols 65h+c = 16*Wv[64h+c, :], col 65h+64 = 0
    WvA = np.zeros((1040, HID), np.float32)
    bvA = np.zeros(1040, np.float32)
    for h in range(NH):
        WvA[65 * h:65 * h + 64] = 16.0 * Wv[64 * h:64 * h + 64]
        bvA[65 * h:65 * h + 64] = 16.0 * bv[64 * h:64 * h + 64]
        bvA[65 * h + 64] = 16.0
    wv8 = np.ascontiguousarray(
        WvA.T.reshape(8, 128, 1040).transpose(1, 0, 2)).astype(E4)

    id8 = np.eye(128, dtype=np.float32).astype(E4)
    ones64 = np.ones((128, 64), np.float32)

    in_maps = []
    for core in range(NCORES):
        b, g = core // 2, core % 2
        l0 = SL * g
        # tables host-shifted by l0 so the kernel's j0 formulas are
        # core-independent: kernel reads drT8[:, (896-128t)+j]; the true
        # offset is 896-l0-128t => shift the reversed table left by l0.
        # dT8: kernel reads dT8[:, (896-128u)+j]; true j0k = l0+896-128u
        # => shift D2 right by -l0 i.e. index + l0.
        # want drT8[jk] = Dr2[jk - l0]  (since true j0q = 896-l0-128t)
        drT = np.zeros((2048, HD), np.float32)
        if l0 == 0:
            drT[:] = Dr2
        else:
            drT[l0:] = Dr2[:2048 - l0]
        dT = np.zeros((2048, HD), np.float32)
        # want dT8[jk] = D2[jk + l0]  (true j0k = l0+896-128u)
        if l0 == 0:
            dT[:] = D2
        else:
            dT[:2048 - l0] = D2[l0:]
        drT8 = np.ascontiguousarray(
            np.tile((8.0 * drT).T, (2, 1))).astype(E4)   # [128, 2048]
        dT8 = np.ascontiguousarray(
            np.tile((8.0 * dT).T, (2, 1))).astype(E4)

        hsT8 = np.ascontiguousarray(
            hs[b].T.reshape(8, 128, S).transpose(1, 0, 2)).astype(E4)
        hsQ8 = np.ascontiguousarray(hsT8[:, :, l0:l0 + SL])
        res = np.ascontiguousarray(hs[b, l0:l0 + SL] + bo[None, :])
        in_maps.append({
            "hsT8": hsT8,
            "hsQ8": hsQ8,
            "res": res,
            "wq8": wq8, "wk8": wk8, "wv8": wv8, "wo8": wo8,
            "bq8": np.ascontiguousarray((8.0 * bq).reshape(8, 128).T),
            "bk8": np.ascontiguousarray((8.0 * bk).reshape(8, 128).T),
            "bv16": bvA,
            "drT8": drT8, "dT8": dT8,
            "id8": id8,
            "maskc": np.ascontiguousarray(mask[b, 0, 0].reshape(8, 128).T),
            "ones64": ones64,
            "lng": ln_g, "lnb": ln_b,
        })
    return in_maps


def kernel(**inputs):
    global _COMPILED
    if _COMPILED is None:
        _COMPILED = build_program()
    nc = _COMPILED
    in_maps = make_in_maps(**inputs)
    result = run_bass_kernel_spmd(nc, in_maps, core_ids=list(range(NCORES)))
    out = np.zeros((B, S, HID), np.float32)
    for core in range(NCORES):
        b, g = core // 2, core % 2
        out[b, SL * g:SL * g + SL] = result.results[core]["out"]
    return out
